# revision 1
# baseline (speedup 1.0000x reference)
"""AriaGroupedGEMM (MoE grouped GEMM) on 8 TRN2 NeuronCores.

Problem: input [4096, 2048] f32, weight [8, 2048, 2048] f32,
tokens_per_expert [8] int32 (tokens pre-sorted by expert).
out[i] = input[i] @ weight[expert_of(i)].

Strategy: expert-parallel. Core g owns expert g's weight and its token
group (boundaries computed on host from tokens_per_expert). Each core
runs a dense [T_pad, 2048] @ [2048, 2048] GEMM in bf16 (fp32 PSUM
accumulation). Host pre-swizzles operands into SBUF-native layouts so
every DMA is fully contiguous, and gathers/unpads the result.
"""
import sys
import functools

for _p in ("/opt/trn_rl_repo", "/root/.axon_site/_ro/trn_rl_repo"):
    if _p not in sys.path:
        sys.path.insert(0, _p)

import numpy as np
import ml_dtypes

import concourse.mybir as mybir
import concourse.tile as tile
from concourse import bacc
from concourse import bass_utils

P = 128
K = 2048            # in_features (contraction)
N = 2048            # out_features
G = 8               # experts == cores
KO = K // P         # 16 k-subtiles
KH = 2              # k halves (DMA granularity on k)
KOH = KO // KH      # 8 k-subtiles per half
NB = N // 512       # 4 n-blocks of 512

COMPUTE_DT = mybir.dt.bfloat16
NP_COMPUTE = ml_dtypes.bfloat16


@functools.lru_cache(maxsize=4)
def _build(t_pad: int):
    """Build + compile the per-core GEMM graph for token-pad t_pad."""
    mt = t_pad // P  # m tiles of 128 tokens

    nc = bacc.Bacc("TRN2", target_bir_lowering=False, debug=False)

    # host-swizzled, fully contiguous per DMA tile:
    # xt[kh, p, ko, m]  = X[m, (kh*KOH+ko)*P + p]
    # w[n, kh, p, ko, j] = W[(kh*KOH+ko)*P + p, n*512 + j]
    xt_d = nc.dram_tensor(
        "xt", [KH, P, KOH, t_pad], COMPUTE_DT, kind="ExternalInput"
    ).ap()
    w_d = nc.dram_tensor(
        "w", [NB, KH, P, KOH, 512], COMPUTE_DT, kind="ExternalInput"
    ).ap()
    out_d = nc.dram_tensor("out", [t_pad, N], mybir.dt.float32, kind="ExternalOutput").ap()

    with tile.TileContext(nc) as tc:
        with (
            tc.tile_pool(name="xt_p", bufs=KH) as xt_p,
            tc.tile_pool(name="w_p", bufs=NB * KH) as w_p,
            tc.tile_pool(name="o_p", bufs=4) as o_p,
            tc.tile_pool(name="ps", bufs=8, space="PSUM") as ps,
        ):
            # stage all input DMAs (issue order == need order)
            xt_t = [None] * KH
            w_t = [[None] * KH for _ in range(NB)]

            def load_xt(kh):
                t = xt_p.tile([P, KOH, t_pad], COMPUTE_DT, tag="xt")
                nc.sync.dma_start(t[:], xt_d[kh])
                xt_t[kh] = t

            def load_w(n, kh):
                t = w_p.tile([P, KOH, 512], COMPUTE_DT, tag="w")
                nc.sync.dma_start(t[:], w_d[n, kh])
                w_t[n][kh] = t

            load_xt(0)
            load_w(0, 0)
            load_xt(1)
            load_w(0, 1)
            for n in range(1, NB):
                for kh in range(KH):
                    load_w(n, kh)

            for n in range(NB):
                for m in range(mt):
                    psum_t = ps.tile([P, 512], mybir.dt.float32)
                    for k in range(KO):
                        kh, ko = divmod(k, KOH)
                        nc.tensor.matmul(
                            psum_t[:],
                            xt_t[kh][:, ko, m * P:(m + 1) * P],
                            w_t[n][kh][:, ko, :],
                            start=(k == 0),
                            stop=(k == KO - 1),
                        )
                    o_sb = o_p.tile([P, 512], mybir.dt.float32, tag="o")
                    nc.vector.tensor_copy(o_sb[:], psum_t[:])
                    nc.scalar.dma_start(
                        out_d[m * P:(m + 1) * P, n * 512:(n + 1) * 512], o_sb[:]
                    )

    nc.compile()
    return nc


def _swizzle_x(x_pad: np.ndarray, t_pad: int) -> np.ndarray:
    # [t_pad, K] f32 -> [KH, P, KOH, t_pad] bf16, xt[kh,p,ko,m] = X[m,(kh*KOH+ko)*P+p]
    v = x_pad.reshape(t_pad, KH, KOH, P).transpose(1, 3, 2, 0)
    return np.ascontiguousarray(v.astype(NP_COMPUTE))


def _swizzle_w(w_g: np.ndarray) -> np.ndarray:
    # [K, N] f32 -> [NB, KH, P, KOH, 512]
    v = w_g.reshape(KH, KOH, P, NB, 512).transpose(3, 0, 2, 1, 4)
    return np.ascontiguousarray(v.astype(NP_COMPUTE))


def _run(input, weight, tokens_per_expert, trace=False, **trace_kwargs):
    inp = np.asarray(input)
    wgt = np.asarray(weight)
    counts = np.asarray(tokens_per_expert).astype(np.int64)
    num_tokens, k = inp.shape
    assert k == K and wgt.shape == (G, K, N)
    ends = np.cumsum(counts)
    starts = ends - counts

    t_pad = max(P, int(-(-counts.max() // P)) * P)
    nc = _build(t_pad)

    in_maps = []
    for g in range(G):
        x_pad = np.zeros((t_pad, K), dtype=np.float32)
        x_pad[: counts[g]] = inp[starts[g]:ends[g]]
        in_maps.append({"xt": _swizzle_x(x_pad, t_pad), "w": _swizzle_w(wgt[g])})

    res = bass_utils.run_bass_kernel_spmd(
        nc, in_maps, core_ids=list(range(G)), trace=trace, **trace_kwargs
    )

    out = np.empty((num_tokens, N), dtype=np.float32)
    for g in range(G):
        out[starts[g]:ends[g]] = res.results[g]["out"][: counts[g]]
    return out, res


def kernel(input, weight, tokens_per_expert):
    out, _ = _run(input, weight, tokens_per_expert)
    return out
